# revision 21
# baseline (speedup 1.0000x reference)
"""Trainium2 Bass kernel: ArgumentRelationAttention.

out[b] = softmax_j(mask_diag(x[b] @ W @ x[b]^T + bias)) @ x[b]
  x: [64, 512, 768] f32, W: [768, 768] f32, bias: [1] f32

Strategy: pure batch data parallelism — 8 batches per NeuronCore x 8 cores.
Per batch everything stays on-chip; the TensorEngine runs ONLY the three
matmul families (f32r score path, bf16 output path):
  xT   = DMA-xbar transpose of x (f32 moved as lo/hi uint16 pairs)
  xWt[k,i] = sum_h W[h,k] xT[h,i]          (36 mm, f32r)
  S    = (xW) @ x^T                        (24 mm, f32r)
  row softmax: fused (S + additive diag/bias mask) + row-max in one DVE
  tensor_tensor_reduce, exp + row-sum in one ScalarE pass (output bf16),
  E^T  = DMA-xbar transpose (bf16),
  out  = diag(1/Z) * E @ x                 (32 mm, bf16), row scale fused
         into the PSUM->SBUF evacuation.

The diagonal is excluded via an additive -30000 mask (the reference scores
the diagonal at exactly 0, whose softmax weight ~e^-40 is far below f32
noise for these score magnitudes). Batches are software-pipelined:
finalize(b-1) is emitted after scores(b) so the PE never goes HAM-cold.
"""

import numpy as np

B, N, H = 64, 512, 768
NCORES = 8
BPC = B // NCORES   # batches per core
NP = 128            # SBUF partitions
NC_I = N // NP      # 4 chunks of the sequence dim
NC_H = H // NP      # 6 chunks of the hidden dim
FH = 384            # mm-C free-dim split (768 = 2*384, <= 512 fp32 PSUM bank)
NEG_BIG = -30000.0

_CACHE = {}


def _build(bpc=BPC, mm_dtype_name="float32r"):
    import concourse.bass as bass  # noqa: F401
    import concourse.tile as tile
    from concourse import bacc, mybir
    from concourse.bass import ts, ds

    f32 = mybir.dt.float32
    bf16 = mybir.dt.bfloat16
    u16 = mybir.dt.uint16
    mdt = getattr(mybir.dt, mm_dtype_name)

    nc = bacc.Bacc(
        "TRN2",
        target_bir_lowering=False,
        debug=False,
        enable_asserts=True,
        num_devices=NCORES,
    )
    x_ext = nc.dram_tensor("arg_embeddings", [bpc, N, H], mdt, kind="ExternalInput").ap()
    w_ext = nc.dram_tensor("relation_W", [H, H], mdt, kind="ExternalInput").ap()
    b_ext = nc.dram_tensor("relation_b", [1, 1], f32, kind="ExternalInput").ap()
    out_ext = nc.dram_tensor("out", [bpc, N, H], f32, kind="ExternalOutput").ap()

    with tile.TileContext(nc) as tc:
        with (
            tc.tile_pool(name="const", bufs=1) as const_pool,
            tc.tile_pool(name="w", bufs=1) as w_pool,
            tc.tile_pool(name="xnat", bufs=4) as xnat_pool,
            tc.tile_pool(name="x16", bufs=3) as x16_pool,
            tc.tile_pool(name="xT", bufs=3 * NC_H) as xT_pool,
            tc.tile_pool(name="xWt", bufs=2 * NC_H) as xWt_pool,
            tc.tile_pool(name="ssb", bufs=3) as s_pool,
            tc.tile_pool(name="e", bufs=2 * NC_I) as e_pool,
            tc.tile_pool(name="et", bufs=2 * NC_I) as et_pool,
            tc.tile_pool(name="stat", bufs=2 * NC_I) as stat_pool,
            tc.tile_pool(name="osb", bufs=NC_I) as out_pool,
            tc.tile_pool(name="psT", bufs=3, space="PSUM") as psT_pool,
            tc.tile_pool(name="psA", bufs=2, space="PSUM") as psA_pool,
            tc.tile_pool(name="psS", bufs=1, space="PSUM") as psS_pool,
            tc.tile_pool(name="psC", bufs=2, space="PSUM") as psC_pool,
        ):
            # identity first — it gates batch 0's transposes
            ident_f32 = const_pool.tile([NP, NP], f32, tag="ident_f32")
            from concourse.masks import make_identity

            make_identity(nc, ident_f32[:])
            ident = const_pool.tile([NP, NP], mdt, tag="ident")
            nc.gpsimd.tensor_copy(out=ident[:], in_=ident_f32[:])
            ident16 = const_pool.tile([NP, NP], bf16, tag="ident16")
            nc.gpsimd.tensor_copy(out=ident16[:], in_=ident_f32[:])

            def emit_load(b):
                x_nat = xnat_pool.tile([NP, NC_I, H], mdt, tag="xnat")
                for ic in range(NC_I):
                    nc.sync.dma_start(x_nat[:, ic, :], x_ext[b][ts(ic, NP), :])

                # x^T chunks via PE transposes, 4 per PSUM bank
                xT = []
                for hc in range(NC_H):
                    pt = psT_pool.tile([NP, N], mdt, tag="psT")
                    for ic in range(NC_I):
                        nc.tensor.matmul(
                            pt[:, ts(ic, NP)],
                            x_nat[:, ic, ts(hc, NP)],
                            ident[:],
                            is_transpose=True,
                            start=(ic == 0),
                            stop=(ic == NC_I - 1),
                        )
                    xt = xT_pool.tile([NP, N], mdt, tag="xT")
                    nc.scalar.copy(out=xt[:], in_=pt[:])
                    xT.append(xt)
                return x_nat, xT

            def emit_consts():
                # additive mask: NEG_BIG on the diagonal, +bias everywhere else
                masks = const_pool.tile([NP, NC_I, N], f32, tag="masks")
                nc.gpsimd.memset(masks[:], 0.0)
                for ic in range(NC_I):
                    nc.gpsimd.affine_select(
                        out=masks[:, ic, :],
                        in_=masks[:, ic, :],
                        compare_op=mybir.AluOpType.not_equal,
                        fill=NEG_BIG,
                        base=ic * NP,
                        channel_multiplier=1,
                        pattern=[[-1, N]],
                    )
                b_row = const_pool.tile([1, 1], f32, tag="brow")
                nc.sync.dma_start(b_row[:], b_ext[:])
                b_col = const_pool.tile([NP, 1], f32, tag="bcol")
                nc.gpsimd.partition_broadcast(b_col[:], b_row[:])
                nc.gpsimd.tensor_scalar_add(masks[:], masks[:], b_col[:])

                w_tile = w_pool.tile([NP, NC_H, H], mdt, tag="w")
                for hc in range(NC_H):
                    nc.sync.dma_start(w_tile[:, hc, :], w_ext[ts(hc, NP), :])
                return masks, w_tile

            C = {}

            def emit_math(b, x_nat, xT):
                w_tile = C["w"]
                masks = C["masks"]
                # xWt[kc][p, i] = sum_h W[h, kc*128+p] * x[i, h]
                xWt = []
                for kc in range(NC_H):
                    ps = psA_pool.tile([NP, N], f32, tag="psA")
                    for hc in range(NC_H):
                        nc.tensor.matmul(
                            ps[:],
                            w_tile[:, hc, ts(kc, NP)],
                            xT[hc][:],
                            start=(hc == 0),
                            stop=(hc == NC_H - 1),
                        )
                    xw = xWt_pool.tile([NP, N], mdt, tag="xWt")
                    nc.vector.tensor_copy(out=xw[:], in_=ps[:])
                    xWt.append(xw)

                # S chunk ic: S[p, j] = sum_k xWt[k, ic*128+p] * xT[k, j]
                E, R = [], []
                for ic in range(NC_I):
                    ps = psS_pool.tile([NP, N], f32, tag="psS")
                    for kc in range(NC_H):
                        nc.tensor.matmul(
                            ps[:],
                            xWt[kc][:, ts(ic, NP)],
                            xT[kc][:],
                            start=(kc == 0),
                            stop=(kc == NC_H - 1),
                        )
                    # ssb = S + mask(bias, diag)
                    ssb = s_pool.tile([NP, N], f32, tag="ssb")
                    nc.vector.tensor_add(ssb[:], ps[:], masks[:, ic, :])
                    negm = stat_pool.tile([NP, 1], f32, tag="negm")
                    nc.vector.tensor_reduce(
                        negm[:],
                        ssb[:],
                        axis=mybir.AxisListType.X,
                        op=mybir.AluOpType.max,
                        negate=True,
                    )
                    e = e_pool.tile([NP, N], bf16, tag="e")
                    z = stat_pool.tile([NP, 1], f32, tag="z")
                    nc.scalar.activation(
                        e[:],
                        ssb[:],
                        mybir.ActivationFunctionType.Exp,
                        bias=negm[:],
                        scale=1.0,
                        accum_out=z[:],
                    )
                    r = stat_pool.tile([NP, 1], f32, tag="r")
                    nc.vector.reciprocal(r[:], z[:])
                    E.append(e)
                    R.append(r)
                return {"x_nat": x_nat, "E": E, "R": R, "b": b}


            def emit_finalize(st):
                b, x_nat, E, R = st["b"], st["x_nat"], st["E"], st["R"]
                x16 = x16_pool.tile([NP, NC_I, H], bf16, tag="x16")
                nc.vector.tensor_copy(out=x16[:], in_=x_nat[:])
                # E^T chunks (bf16) via PE transposes, 4 per PSUM bank
                ET = []
                for jc in range(NC_I):
                    pt16 = psT_pool.tile([NP, N], bf16, tag="psT")
                    for ic in range(NC_I):
                        nc.tensor.matmul(
                            pt16[:, ts(ic, NP)],
                            E[ic][:, ts(jc, NP)],
                            ident16[:],
                            is_transpose=True,
                            start=(ic == 0),
                            stop=(ic == NC_I - 1),
                        )
                    et = et_pool.tile([NP, N], bf16, tag="et")
                    nc.vector.tensor_copy(out=et[:], in_=pt16[:])
                    ET.append(et)

                # out chunk ic: out[p, h] = r[p] * sum_j E[ic*128+p, j] x[j, h]
                for ic in range(NC_I):
                    osb = out_pool.tile([NP, H], f32, tag="osb")
                    for nh in range(2):
                        ps = psC_pool.tile([NP, FH], f32, tag="psC")
                        for jc in range(NC_I):
                            nc.tensor.matmul(
                                ps[:],
                                ET[jc][:, ts(ic, NP)],
                                x16[:, jc, ds(nh * FH, FH)],
                                start=(jc == 0),
                                stop=(jc == NC_I - 1),
                            )
                        nc.scalar.activation(
                            osb[:, ds(nh * FH, FH)],
                            ps[:],
                            mybir.ActivationFunctionType.Copy,
                            scale=R[ic][:],
                        )
                    nc.sync.dma_start(out_ext[b][ts(ic, NP), :], osb[:])

            # batch 0's x load + transposes get DMA priority over W/masks;
            # loads run one batch ahead of math so transposes fill PE gaps
            loads = {0: emit_load(0)}
            C["masks"], C["w"] = emit_consts()
            if bpc > 1:
                loads[1] = emit_load(1)
            prev = None
            for b in range(bpc):
                st = emit_math(b, *loads.pop(b))
                if b + 2 < bpc:
                    loads[b + 2] = emit_load(b + 2)
                if prev is not None:
                    emit_finalize(prev)
                prev = st
            emit_finalize(prev)

    nc.compile()
    return nc


def _get_nc(bpc=BPC, mm_dtype_name="float32r"):
    key = (bpc, mm_dtype_name)
    if key not in _CACHE:
        _CACHE[key] = _build(bpc, mm_dtype_name)
    return _CACHE[key]


def make_in_maps(arg_embeddings, relation_W, relation_b, bpc=BPC):
    x = np.ascontiguousarray(arg_embeddings, dtype=np.float32)
    W = np.ascontiguousarray(relation_W, dtype=np.float32)
    bb = np.asarray(relation_b, dtype=np.float32).reshape(1, 1)
    return [
        {
            "arg_embeddings": np.ascontiguousarray(x[c * bpc : (c + 1) * bpc]),
            "relation_W": W,
            "relation_b": bb,
        }
        for c in range(NCORES)
    ]


def kernel(arg_embeddings, relation_W, relation_b):
    from concourse.bass_utils import run_bass_kernel_spmd

    nc = _get_nc()
    in_maps = make_in_maps(arg_embeddings, relation_W, relation_b)
    res = run_bass_kernel_spmd(nc, in_maps, core_ids=list(range(NCORES)))
    out = np.concatenate([res.results[c]["out"] for c in range(NCORES)], axis=0)
    return np.ascontiguousarray(out, dtype=np.float32)
